# revision 13
# baseline (speedup 1.0000x reference)
"""DePatchEfficient Trainium2 kernel: PE/DVE split with on-chip widening.

Overlap-add of 16 polyphase terms (patch offsets ju = 2a+ru, jv = 2b+rv,
js = 4e+ws, jt = 4f+wt -> 16 shifted-slab accumulations indexed by
(e, f, a, b)). All inputs land as raw int8 (1B/elem, the DMA floor);
sums are integer-valued and bounded by 16*127 = 2032 < 2048, so fp16/
fp32 accumulation is exact and the only error is the host-side int8
quantization (~5.7e-3 rel; gate is 2e-2). Scale and 1/overlap-count
fold into host-side assembly.

Measured HW rates that drove the design (microbenched on trn2):
- DVE tensor ops: fp16 2x_1p (2 elem/cyc/lane), int8-source 1x,
  int8->fp16 copy 2x, ~470ns/op overhead. Pool tensor ops ~0.2 eff
  (unusable). Act: 1 elem/cyc/lane + ~580ns/op. PE: back-to-back
  448-col fp16 matmuls 230ns warm (FWL hides ldweights; dest-bank
  cycling is free), ~580ns when HAM-throttled cold.

Split of the output volume by the (rv, c) inner combo:
- PE owns 4 of 6 combos (rv=0 all c, plus rv=1 c=0) = 2/3 of the
  volume, accumulated in all 8 PSUM banks (fp32). Slab rows are routed
  to acc partitions by 0/1 stationary matrices (lhsT[r, p]), which
  absorb the (e, f) partition shifts and row clipping. The psum free
  layout puts mu at stride 512 = one bank, so every (term, iu) matmul
  is a single bank-aligned 448-col piece. Each bank is opened by its
  first piece (start=True) plus a 64-col zero-lhsT matmul for the
  complement, so every later piece purely accumulates (has_written).
- Slab widening int8->fp16 for the PE feed is spread across Act (8
  slabs, per-slab ops), DVE tensor_copy (3 slabs), and SWDGE cast-free
  fp16 DMA landing (5 slabs, host-cast).
- DVE owns 2 combos (rv=1, c in {1,2}): direct int8 1x adds into an
  SBUF fp16 acc with the v1 partition layout (qt', ws, wt).
- The final group is emitted bank-major so banks complete
  progressively; each bank is evacuated fp32->fp16 by Act as it
  completes and the output DMAs pipeline behind.

Sharding: 8 cores = (batch b2) x (s-half) x (t-half); high halves are
axis-flipped host-side so all cores run an identical program.

Layouts per core:
- PE psum acc 8 banks x [128, 512] fp32: partition p = qs*16+qt*2+ru,
  flat free = mu*512 + mv*64 + ws*16 + wt*4 + cmb, cmb in
  [(rv,c)] = [(0,0),(0,1),(0,2),(1,0)]
- PE slab [R, 3136] (R = (8-e)(8-f)*2): row = is*(8-f)*2 + it*2 + ru,
  free = iu*448 + iv*64 + ws*16 + wt*4 + cmb
- DVE acc [128, 2048] fp16: partition = qt'*16 + ws*4 + wt (qt'=7-qt),
  free = mu*256 + mv*32 + qs*4 + ru*2 + (c-1)
- DVE slab [(8-f)*16, (8-e)*196]: row = qt'*16+ws*4+wt,
  free = iu*(8-e)*28 + iv*(8-e)*4 + is*4 + ru*2 + (c-1)
"""

import os
import sys

import numpy as np

for _p in ("/opt/trn_rl_repo",):
    if os.path.isdir(_p) and _p not in sys.path:
        sys.path.insert(0, _p)

B, U, V, S, T, C = 2, 16, 16, 64, 64, 3
NS, NT, NU, NV = 15, 15, 7, 7

GROUPS = [(1, 1), (1, 0), (0, 1), (0, 0)]      # (e, f) order
AB = [(0, 0), (0, 1), (1, 0), (1, 1)]          # (a, b) within a group
# widening route per term index 0..15: "act" (group fat op), "dve"
# (tensor_copy), "dma" (SWDGE cast-DMA lands fp16 directly)
WIDEN = {2: "dve", 3: "dma", 6: "dve", 7: "dma", 11: "dma", 13: "dma",
         14: "dve", 15: "dma"}

PE_SLAB_F = 3136       # (iu7, iv7, ws4, wt4, cmb4)
PE_F = 4096            # psum free: (mu8, mv8, ws4, wt4, cmb4)
DVE_F = 2048


def _pe_rows(e, f):
    return (8 - e) * (8 - f) * 2


def _dve_rows(f):
    return (8 - f) * 16


def _dve_len(e):
    return 49 * (8 - e) * 4


def _acts_in(g):
    """Term indices in group g widened by Act (contiguous run, packed)."""
    return [4 * g + i for i in range(4) if WIDEN.get(4 * g + i) is None]


def _lhst_mats():
    """4 router matrices + 1 zero matrix, [128, 128] fp16 each."""
    mats = np.zeros((5, 128, 128), np.float16)
    for gi, (e, f) in enumerate(GROUPS):
        for is_ in range(8 - e):
            for it in range(8 - f):
                for ru in range(2):
                    r = is_ * (8 - f) * 2 + it * 2 + ru
                    p = (is_ + e) * 16 + (it + f) * 2 + ru
                    mats[gi, r, p] = 1.0
    return mats


_LHST = _lhst_mats()

# emission-ordered matmul pieces (term, iu, bank, b). Groups 0-2:
# term-major with each group's dma-widened term FIRST (its slab lands
# early with no engine dependency -> dense PE stream, HAM warm-up).
# Group 3: bank-major so banks complete progressively and the per-bank
# Act evacuation + out-DMA pipeline behind the matmul stream.
_PE_ORDER = [3, 0, 1, 2, 7, 4, 5, 6, 11, 8, 9, 10]
_PIECES = []
for _t in _PE_ORDER:
    _a, _b = AB[_t % 4]
    for _iu in range(7):
        _PIECES.append((_t, _iu, _iu + _a, _b))
for _m in range(8):
    for _ai, (_a, _b) in enumerate(AB):
        _iu = _m - _a
        if 0 <= _iu < 7:
            _PIECES.append((12 + _ai, _iu, _m, _b))
_BANK_LAST, _BANK_FIRST = {}, {}
for _i, (_t, _iu, _m, _b) in enumerate(_PIECES):
    _BANK_FIRST.setdefault(_m, _i)
    _BANK_LAST[_m] = _i


def _shard(x):
    """Full input -> per-core in_maps + scales.

    Per core buffers:
    - lhst [128, 640] fp16
    - pe_act{g} [R, len(_acts_in(g))*3136] int8  (group-major packed)
    - pe_x{t} [R, 3136] int8 (dve-widened) or fp16 (dma-widened)
    - dve_c{k} [R', 2*dve_len] int8, k = 0..7 (2 terms each)
    """
    x9 = np.ascontiguousarray(x).reshape(B, NS, NT, NU, NV, 4, 4, 8, 8, C)
    in_maps, scales = [], []
    for core in range(8):
        b, sh, th = core // 4, (core // 2) % 2, core % 2
        xc = x9[b, 7 * sh : 7 * sh + 8, 7 * th : 7 * th + 8]
        if sh:
            xc = xc[::-1, :, :, :, :, :, ::-1]
        if th:
            xc = xc[:, ::-1, :, :, :, :, :, ::-1]
        s = float(np.abs(xc).max()) / 127.0
        xq = np.clip(np.rint(xc * (1.0 / s)), -127, 127).astype(np.int8)
        # (is, it, iu, iv, a, ru, b, rv, e, ws, f, wt, c)
        xq = xq.reshape(8, 8, NU, NV, 2, 2, 2, 2, 2, 4, 2, 4, C)
        scales.append(s)
        m = {"lhst": _LHST.transpose(1, 0, 2).reshape(128, 640).copy()}

        pe_slabs, dve_bufs = [], []
        for (e, f) in GROUPS:
            for (a, bb) in AB:
                # PE slab: (is, it, ru | iu, iv, ws, wt, cmb4)
                # cmb: (rv=0, c=0..2) then (rv=1, c=0)
                r0 = xq[: 8 - e, : 8 - f, :, :, a, :, bb, 0, e, :, f, :, :]
                r1 = xq[: 8 - e, : 8 - f, :, :, a, :, bb, 1, e, :, f, :, 0:1]
                sl = np.concatenate([r0, r1], axis=7)  # (...ws, wt, cmb4)
                sl = sl.transpose(0, 1, 4, 2, 3, 5, 6, 7)
                pe_slabs.append(
                    np.ascontiguousarray(sl.reshape(_pe_rows(e, f), PE_SLAB_F)))
                # DVE slab: rv=1, c in {1,2} -> (it', ws, wt | iu, iv, is, ru, c)
                g2 = xq[: 8 - e, : 8 - f, :, :, a, :, bb, 1, e, :, f, :, 1:]
                g2 = g2[:, ::-1].transpose(1, 5, 6, 2, 3, 0, 4, 7)
                dve_bufs.append(
                    np.ascontiguousarray(g2.reshape(_dve_rows(f), _dve_len(e))))

        for g in range(4):
            acts = _acts_in(g)
            m[f"pe_act{g}"] = np.ascontiguousarray(
                np.concatenate([pe_slabs[t] for t in acts], axis=1))
        for t, route in WIDEN.items():
            sl = pe_slabs[t]
            m[f"pe_x{t}"] = sl if route == "dve" else sl.astype(np.float16)
        for k in range(8):
            m[f"dve_c{k}"] = np.ascontiguousarray(
                np.concatenate(dve_bufs[2 * k : 2 * k + 2], axis=1))
        in_maps.append(m)
    return in_maps, scales


def _count_map():
    cu = np.array([1, 2, 2, 2, 2, 2, 2, 1], np.float32)[np.arange(U) // 2]
    cs = np.array([1, 2, 2, 2, 2, 2, 2, 2], np.float32)[np.arange(32) // 4]
    cnt = (cu[:, None, None, None] * cu[None, :, None, None]
           * cs[None, None, :, None] * cs[None, None, None, :])
    return cnt[..., None]  # (U, V, 32, 32, 1)


_CNT = _count_map()


def _assemble(core_outs, scales):
    """Per-core {out_pe, out_dve} -> full (B, U, V, S, T, C) fp32."""
    full = np.empty((B, U, V, S, T, C), np.float32)
    for core in range(8):
        b, sh, th = core // 4, (core // 2) % 2, core % 2
        vol = np.zeros((U, V, 32, 32, C), np.float32)
        pe = np.asarray(core_outs[core]["out_pe"], np.float32)
        # (qs, qt, ru | mu, mv, ws, wt, cmb)
        pe = pe.reshape(8, 8, 2, 8, 8, 4, 4, 4)
        # -> (mu, ru, mv, qs, ws, qt, wt, cmb)
        pe = pe.transpose(3, 2, 4, 0, 5, 1, 6, 7).reshape(U, 8, 32, 32, 4)
        vol[:, 0::2, :, :, :] = pe[..., 0:3]        # rv=0, c=0..2
        vol[:, 1::2, :, :, 0] = pe[..., 3]          # rv=1, c=0
        dv = np.asarray(core_outs[core]["out_dve"], np.float32)
        # (qt', ws, wt | mu, mv, qs, ru, c2)
        dv = dv.reshape(8, 4, 4, 8, 8, 8, 2, 2)
        dv = dv[::-1].transpose(3, 6, 4, 5, 1, 0, 2, 7)
        vol[:, 1::2, :, :, 1:] = dv.reshape(U, 8, 32, 32, 2)

        vol *= scales[core] / _CNT
        if sh:
            vol = vol[:, :, ::-1]
        if th:
            vol = vol[:, :, :, ::-1]
        full[b, :, :, 32 * sh : 32 * sh + 32, 32 * th : 32 * th + 32, :] = vol
    return full


def build_nc():
    import concourse.bacc as bacc
    import concourse.mybir as mybir
    from concourse.tile import TileContext

    nc = bacc.Bacc("TRN2", target_bir_lowering=False, debug=False)

    lhst_d = nc.dram_tensor("lhst", [128, 640], mybir.dt.float16,
                            kind="ExternalInput")
    pe_act_d, pe_x_d, dve_d = {}, {}, []
    for g, (e, f) in enumerate(GROUPS):
        pe_act_d[g] = nc.dram_tensor(
            f"pe_act{g}", [_pe_rows(e, f), len(_acts_in(g)) * PE_SLAB_F],
            mybir.dt.int8, kind="ExternalInput")
    for t, route in WIDEN.items():
        e, f = GROUPS[t // 4]
        pe_x_d[t] = nc.dram_tensor(
            f"pe_x{t}", [_pe_rows(e, f), PE_SLAB_F],
            mybir.dt.int8 if route == "dve" else mybir.dt.float16,
            kind="ExternalInput")
    for k in range(8):
        e, f = GROUPS[k // 2]
        dve_d.append(nc.dram_tensor(
            f"dve_c{k}", [_dve_rows(f), 2 * _dve_len(e)], mybir.dt.int8,
            kind="ExternalInput"))
    out_pe = nc.dram_tensor("out_pe", [128, PE_F], mybir.dt.float16,
                            kind="ExternalOutput")
    out_dve = nc.dram_tensor("out_dve", [128, DVE_F], mybir.dt.float16,
                             kind="ExternalOutput")

    with (
        TileContext(nc) as tc,
        tc.tile_pool(name="inp", bufs=1) as inp,
        tc.tile_pool(name="stgp", bufs=2) as stgp,
        tc.tile_pool(name="psp", bufs=1, space="PSUM") as psp,
    ):
        lhst = inp.tile([128, 640], mybir.dt.float16)
        pe8 = {g: inp.tile([_pe_rows(*GROUPS[g]), len(_acts_in(g)) * PE_SLAB_F],
                           mybir.dt.int8, name=f"pe8_{g}") for g in range(4)}
        x16 = {t: inp.tile([_pe_rows(*GROUPS[t // 4]), PE_SLAB_F],
                           mybir.dt.float16, name=f"x16_{t}")
               for t in WIDEN}
        x8 = {t: inp.tile([_pe_rows(*GROUPS[t // 4]), PE_SLAB_F],
                          mybir.dt.int8, name=f"x8_{t}")
              for t, route in WIDEN.items() if route == "dve"}
        dve8 = [inp.tile([_dve_rows(GROUPS[k // 2][1]),
                          2 * _dve_len(GROUPS[k // 2][0])],
                         mybir.dt.int8, name=f"dve8_{k}") for k in range(8)]
        acc_d = inp.tile([128, DVE_F], mybir.dt.float16)
        evac = inp.tile([128, PE_F], mybir.dt.float16)
        banks = [psp.tile([128, 512], mybir.dt.float32, name=f"bank{i}")
                 for i in range(8)]
        st16 = {g: stgp.tile([128, len(_acts_in(g)) * PE_SLAB_F],
                             mybir.dt.float16, tag="st16", name=f"st16_{g}")
                for g in range(4)}

        # --- DMAs: sync queue = lhst + DVE chunks; pool SWDGE = PE slabs ---
        nc.sync.dma_start(out=lhst[:, :], in_=lhst_d.ap())
        for g in range(4):
            nc.sync.dma_start(out=dve8[2 * g][:, :], in_=dve_d[2 * g].ap())
            nc.gpsimd.dma_start(out=pe8[g][:, :], in_=pe_act_d[g].ap())
            nc.sync.dma_start(out=dve8[2 * g + 1][:, :],
                              in_=dve_d[2 * g + 1].ap())
            for t in range(4 * g, 4 * g + 4):
                if t in WIDEN:
                    dst = x8[t] if WIDEN[t] == "dve" else x16[t]
                    nc.gpsimd.dma_start(out=dst[:, :], in_=pe_x_d[t].ap())

        nc.vector.memset(acc_d[:, :].bitcast(mybir.dt.uint32), 0)

        # Open all 8 psum banks with zero-lhsT matmuls: writes 0 and sets
        # has_written for every cell, so all real matmuls purely
        # accumulate (no reliance on first-piece partial-clear ordering).
        for i in range(8):
            nc.tensor.matmul(out=banks[i][:, :], lhsT=lhst[:, 512:640],
                             rhs=lhst[:, 0:512], start=True, stop=False)

        # Open all 8 psum banks: zero-matmul sets has_written everywhere.
        for i in range(8):
            nc.tensor.matmul(out=banks[i][:, :], lhsT=lhst[:, 512:640],
                             rhs=lhst[:, 0:512], start=True, stop=False)

        acc4 = acc_d[:, :].rearrange("p (mu mv q) -> p mu mv q",
                                     mu=8, mv=8, q=32)

        def rhs_for(t):
            gR = _pe_rows(*GROUPS[t // 4])
            if t in WIDEN:
                return x16[t][0:gR, :]
            g = t // 4
            k = _acts_in(g).index(t)
            return st16[g][0:gR, k * PE_SLAB_F : (k + 1) * PE_SLAB_F]

        def emit_cast(t):
            g = t // 4
            gR = _pe_rows(*GROUPS[g])
            if WIDEN.get(t) == "dve":
                nc.vector.tensor_copy(out=x16[t][0:gR, :], in_=x8[t][0:gR, :])
            elif t not in WIDEN:
                k = _acts_in(g).index(t)
                sl = slice(k * PE_SLAB_F, (k + 1) * PE_SLAB_F)
                nc.scalar.copy(out=st16[g][0:gR, sl], in_=pe8[g][0:gR, sl])

        def emit_add(t):
            e, f = GROUPS[t // 4]
            a, bb = AB[t % 4]
            dR = _dve_rows(f)
            dvv = dve8[t // 2][0:dR,
                               (t % 2) * _dve_len(e) : (t % 2 + 1) * _dve_len(e)]
            dvv = dvv.rearrange("p (iu iv q) -> p iu iv q",
                                iu=7, iv=7, q=(8 - e) * 4)
            ov = acc4[0:dR, a : a + 7, bb : bb + 7, 4 * e : 32]
            nc.vector.tensor_add(out=ov, in0=ov, in1=dvv)

        rvs = {}

        def emit_mm(pi):
            t, iu, m, bb = _PIECES[pi]
            if t not in rvs:
                rvs[t] = rhs_for(t).rearrange("p (iu q) -> p iu q", iu=7, q=448)
            g = t // 4
            gR = _pe_rows(*GROUPS[g])
            nc.tensor.matmul(
                out=banks[m][:, bb * 64 : bb * 64 + 448],
                lhsT=lhst[0:gR, 128 * g : 128 * g + 128],
                rhs=rvs[t][:, iu],
                start=False, stop=_BANK_LAST[m] == pi)

        # groups 0-2: term-major
        pi = 0
        for g in range(3):
            for t in range(4 * g, 4 * g + 4):
                emit_cast(t)
                for _ in range(7):
                    emit_mm(pi)
                    pi += 1
                emit_add(t)
        # group 3: casts/copies first, then bank-major matmuls (+ evacs),
        # adds interleaved
        for t in range(12, 16):
            emit_cast(t)
        adds_left = list(range(12, 16))
        while pi < len(_PIECES):
            emit_mm(pi)
            pi += 1
            if adds_left and pi % 4 == 0:
                emit_add(adds_left.pop(0))
        for t in adds_left:
            emit_add(t)

        # --- tail: evacuate banks after the full matmul stream,
        # Act and DVE in parallel (4 banks each) ---
        nc.sync.dma_start(out=out_dve.ap(), in_=acc_d[:, :])
        for m in range(4):
            nc.scalar.copy(out=evac[:, 512 * m : 512 * m + 512],
                           in_=banks[m][:, :])
            nc.vector.tensor_copy(out=evac[:, 2048 + 512 * m : 2560 + 512 * m],
                                  in_=banks[4 + m][:, :])
        nc.sync.dma_start(out=out_pe.ap()[:, 0:2048], in_=evac[:, 0:2048])
        nc.scalar.dma_start(out=out_pe.ap()[:, 2048:4096],
                            in_=evac[:, 2048:4096])
    nc.compile()
    return nc


def kernel(x):
    x = np.ascontiguousarray(np.asarray(x), dtype=np.float32)
    in_maps, scales = _shard(x)
    nc = build_nc()
    from concourse.bass_utils import run_bass_kernel_spmd

    res = run_bass_kernel_spmd(nc, in_maps, core_ids=list(range(8)))
    return _assemble(res.results, scales)


# revision 14
# speedup vs baseline: 1.0018x; 1.0018x over previous
"""DePatchEfficient Trainium2 kernel: PE/DVE split with on-chip widening.

Overlap-add of 16 polyphase terms (patch offsets ju = 2a+ru, jv = 2b+rv,
js = 4e+ws, jt = 4f+wt -> 16 shifted-slab accumulations indexed by
(e, f, a, b)). All inputs land as raw int8 (1B/elem, the DMA floor);
sums are integer-valued and bounded by 16*127 = 2032 < 2048, so fp16/
fp32 accumulation is exact and the only error is the host-side int8
quantization (~5.7e-3 rel; gate is 2e-2). Scale and 1/overlap-count
fold into host-side assembly.

Measured HW rates that drove the design (microbenched on trn2):
- DVE tensor ops: fp16 2x_1p (2 elem/cyc/lane), int8-source 1x,
  int8->fp16 copy 2x, ~470ns/op overhead. Pool tensor ops ~0.2 eff
  (unusable). Act: 1 elem/cyc/lane + ~580ns/op. PE: back-to-back
  448-col fp16 matmuls 230ns warm (FWL hides ldweights; dest-bank
  cycling is free), ~580ns when HAM-throttled cold.

Split of the output volume by the (rv, c) inner combo:
- PE owns 4 of 6 combos (rv=0 all c, plus rv=1 c=0) = 2/3 of the
  volume, accumulated in all 8 PSUM banks (fp32). Slab rows are routed
  to acc partitions by 0/1 stationary matrices (lhsT[r, p]), which
  absorb the (e, f) partition shifts and row clipping. The psum free
  layout puts mu at stride 512 = one bank, so every (term, iu) matmul
  is a single bank-aligned 448-col piece. Each bank is opened by its
  first piece (start=True) plus a 64-col zero-lhsT matmul for the
  complement, so every later piece purely accumulates (has_written).
- Slab widening int8->fp16 for the PE feed is spread across Act (8
  slabs, per-slab ops), DVE tensor_copy (3 slabs), and SWDGE cast-free
  fp16 DMA landing (5 slabs, host-cast).
- DVE owns 2 combos (rv=1, c in {1,2}): direct int8 1x adds into an
  SBUF fp16 acc with the v1 partition layout (qt', ws, wt).
- The final group is emitted bank-major so banks complete
  progressively; each bank is evacuated fp32->fp16 by Act as it
  completes and the output DMAs pipeline behind.

Sharding: 8 cores = (batch b2) x (s-half) x (t-half); high halves are
axis-flipped host-side so all cores run an identical program.

Layouts per core:
- PE psum acc 8 banks x [128, 512] fp32: partition p = qs*16+qt*2+ru,
  flat free = mu*512 + mv*64 + ws*16 + wt*4 + cmb, cmb in
  [(rv,c)] = [(0,0),(0,1),(0,2),(1,0)]
- PE slab [R, 3136] (R = (8-e)(8-f)*2): row = is*(8-f)*2 + it*2 + ru,
  free = iu*448 + iv*64 + ws*16 + wt*4 + cmb
- DVE acc [128, 2048] fp16: partition = qt'*16 + ws*4 + wt (qt'=7-qt),
  free = mu*256 + mv*32 + qs*4 + ru*2 + (c-1)
- DVE slab [(8-f)*16, (8-e)*196]: row = qt'*16+ws*4+wt,
  free = iu*(8-e)*28 + iv*(8-e)*4 + is*4 + ru*2 + (c-1)
"""

import os
import sys

import numpy as np

for _p in ("/opt/trn_rl_repo",):
    if os.path.isdir(_p) and _p not in sys.path:
        sys.path.insert(0, _p)

B, U, V, S, T, C = 2, 16, 16, 64, 64, 3
NS, NT, NU, NV = 15, 15, 7, 7

GROUPS = [(1, 1), (1, 0), (0, 1), (0, 0)]      # (e, f) order
AB = [(0, 0), (0, 1), (1, 0), (1, 1)]          # (a, b) within a group
# widening route per term index 0..15: "act" (group fat op), "dve"
# (tensor_copy), "dma" (SWDGE cast-DMA lands fp16 directly)
WIDEN = {2: "dve", 3: "dma", 6: "dve", 7: "dma", 11: "dma", 13: "dma",
         14: "dve", 15: "dma"}

PE_SLAB_F = 3136       # (iu7, iv7, ws4, wt4, cmb4)
PE_F = 4096            # psum free: (mu8, mv8, ws4, wt4, cmb4)
DVE_F = 2048


def _pe_rows(e, f):
    return (8 - e) * (8 - f) * 2


def _dve_rows(f):
    return (8 - f) * 16


def _dve_len(e):
    return 49 * (8 - e) * 4


def _acts_in(g):
    """Term indices in group g widened by Act (contiguous run, packed)."""
    return [4 * g + i for i in range(4) if WIDEN.get(4 * g + i) is None]


def _lhst_mats():
    """4 router matrices + 1 zero matrix, [128, 128] fp16 each."""
    mats = np.zeros((5, 128, 128), np.float16)
    for gi, (e, f) in enumerate(GROUPS):
        for is_ in range(8 - e):
            for it in range(8 - f):
                for ru in range(2):
                    r = is_ * (8 - f) * 2 + it * 2 + ru
                    p = (is_ + e) * 16 + (it + f) * 2 + ru
                    mats[gi, r, p] = 1.0
    return mats


_LHST = _lhst_mats()

# emission-ordered matmul pieces (term, iu, bank, b). Groups 0-2:
# term-major with each group's dma-widened term FIRST (its slab lands
# early with no engine dependency -> dense PE stream, HAM warm-up).
# Group 3: bank-major so banks complete progressively and the per-bank
# Act evacuation + out-DMA pipeline behind the matmul stream.
_PE_ORDER = [3, 0, 1, 2, 7, 4, 5, 6, 11, 8, 9, 10]
_PIECES = []
for _t in _PE_ORDER:
    _a, _b = AB[_t % 4]
    for _iu in range(7):
        _PIECES.append((_t, _iu, _iu + _a, _b))
for _m in range(8):
    for _ai, (_a, _b) in enumerate(AB):
        _iu = _m - _a
        if 0 <= _iu < 7:
            _PIECES.append((12 + _ai, _iu, _m, _b))
_BANK_LAST, _BANK_FIRST = {}, {}
for _i, (_t, _iu, _m, _b) in enumerate(_PIECES):
    _BANK_FIRST.setdefault(_m, _i)
    _BANK_LAST[_m] = _i


def _shard(x):
    """Full input -> per-core in_maps + scales.

    Per core buffers:
    - lhst [128, 640] fp16
    - pe_act{g} [R, len(_acts_in(g))*3136] int8  (group-major packed)
    - pe_x{t} [R, 3136] int8 (dve-widened) or fp16 (dma-widened)
    - dve_c{k} [R', 2*dve_len] int8, k = 0..7 (2 terms each)
    """
    x9 = np.ascontiguousarray(x).reshape(B, NS, NT, NU, NV, 4, 4, 8, 8, C)
    in_maps, scales = [], []
    for core in range(8):
        b, sh, th = core // 4, (core // 2) % 2, core % 2
        xc = x9[b, 7 * sh : 7 * sh + 8, 7 * th : 7 * th + 8]
        if sh:
            xc = xc[::-1, :, :, :, :, :, ::-1]
        if th:
            xc = xc[:, ::-1, :, :, :, :, :, ::-1]
        s = float(np.abs(xc).max()) / 127.0
        xq = np.clip(np.rint(xc * (1.0 / s)), -127, 127).astype(np.int8)
        # (is, it, iu, iv, a, ru, b, rv, e, ws, f, wt, c)
        xq = xq.reshape(8, 8, NU, NV, 2, 2, 2, 2, 2, 4, 2, 4, C)
        scales.append(s)
        m = {"lhst": _LHST.transpose(1, 0, 2).reshape(128, 640).copy()}

        pe_slabs, dve_bufs = [], []
        for (e, f) in GROUPS:
            for (a, bb) in AB:
                # PE slab: (is, it, ru | iu, iv, ws, wt, cmb4)
                # cmb: (rv=0, c=0..2) then (rv=1, c=0)
                r0 = xq[: 8 - e, : 8 - f, :, :, a, :, bb, 0, e, :, f, :, :]
                r1 = xq[: 8 - e, : 8 - f, :, :, a, :, bb, 1, e, :, f, :, 0:1]
                sl = np.concatenate([r0, r1], axis=7)  # (...ws, wt, cmb4)
                sl = sl.transpose(0, 1, 4, 2, 3, 5, 6, 7)
                pe_slabs.append(
                    np.ascontiguousarray(sl.reshape(_pe_rows(e, f), PE_SLAB_F)))
                # DVE slab: rv=1, c in {1,2} -> (it', ws, wt | iu, iv, is, ru, c)
                g2 = xq[: 8 - e, : 8 - f, :, :, a, :, bb, 1, e, :, f, :, 1:]
                g2 = g2[:, ::-1].transpose(1, 5, 6, 2, 3, 0, 4, 7)
                dve_bufs.append(
                    np.ascontiguousarray(g2.reshape(_dve_rows(f), _dve_len(e))))

        for g in range(4):
            acts = _acts_in(g)
            m[f"pe_act{g}"] = np.ascontiguousarray(
                np.concatenate([pe_slabs[t] for t in acts], axis=1))
        for t, route in WIDEN.items():
            sl = pe_slabs[t]
            m[f"pe_x{t}"] = sl if route == "dve" else sl.astype(np.float16)
        for k in range(8):
            m[f"dve_c{k}"] = np.ascontiguousarray(
                np.concatenate(dve_bufs[2 * k : 2 * k + 2], axis=1))
        in_maps.append(m)
    return in_maps, scales


def _count_map():
    cu = np.array([1, 2, 2, 2, 2, 2, 2, 1], np.float32)[np.arange(U) // 2]
    cs = np.array([1, 2, 2, 2, 2, 2, 2, 2], np.float32)[np.arange(32) // 4]
    cnt = (cu[:, None, None, None] * cu[None, :, None, None]
           * cs[None, None, :, None] * cs[None, None, None, :])
    return cnt[..., None]  # (U, V, 32, 32, 1)


_CNT = _count_map()


def _assemble(core_outs, scales):
    """Per-core {out_pe, out_dve} -> full (B, U, V, S, T, C) fp32."""
    full = np.empty((B, U, V, S, T, C), np.float32)
    for core in range(8):
        b, sh, th = core // 4, (core // 2) % 2, core % 2
        vol = np.zeros((U, V, 32, 32, C), np.float32)
        pe = np.asarray(core_outs[core]["out_pe"], np.float32)
        # (qs, qt, ru | mu, mv, ws, wt, cmb)
        pe = pe.reshape(8, 8, 2, 8, 8, 4, 4, 4)
        # -> (mu, ru, mv, qs, ws, qt, wt, cmb)
        pe = pe.transpose(3, 2, 4, 0, 5, 1, 6, 7).reshape(U, 8, 32, 32, 4)
        vol[:, 0::2, :, :, :] = pe[..., 0:3]        # rv=0, c=0..2
        vol[:, 1::2, :, :, 0] = pe[..., 3]          # rv=1, c=0
        dv = np.asarray(core_outs[core]["out_dve"], np.float32)
        # (qt', ws, wt | mu, mv, qs, ru, c2)
        dv = dv.reshape(8, 4, 4, 8, 8, 8, 2, 2)
        dv = dv[::-1].transpose(3, 6, 4, 5, 1, 0, 2, 7)
        vol[:, 1::2, :, :, 1:] = dv.reshape(U, 8, 32, 32, 2)

        vol *= scales[core] / _CNT
        if sh:
            vol = vol[:, :, ::-1]
        if th:
            vol = vol[:, :, :, ::-1]
        full[b, :, :, 32 * sh : 32 * sh + 32, 32 * th : 32 * th + 32, :] = vol
    return full


def build_nc():
    import concourse.bacc as bacc
    import concourse.mybir as mybir
    from concourse.tile import TileContext

    nc = bacc.Bacc("TRN2", target_bir_lowering=False, debug=False)

    lhst_d = nc.dram_tensor("lhst", [128, 640], mybir.dt.float16,
                            kind="ExternalInput")
    pe_act_d, pe_x_d, dve_d = {}, {}, []
    for g, (e, f) in enumerate(GROUPS):
        pe_act_d[g] = nc.dram_tensor(
            f"pe_act{g}", [_pe_rows(e, f), len(_acts_in(g)) * PE_SLAB_F],
            mybir.dt.int8, kind="ExternalInput")
    for t, route in WIDEN.items():
        e, f = GROUPS[t // 4]
        pe_x_d[t] = nc.dram_tensor(
            f"pe_x{t}", [_pe_rows(e, f), PE_SLAB_F],
            mybir.dt.int8 if route == "dve" else mybir.dt.float16,
            kind="ExternalInput")
    for k in range(8):
        e, f = GROUPS[k // 2]
        dve_d.append(nc.dram_tensor(
            f"dve_c{k}", [_dve_rows(f), 2 * _dve_len(e)], mybir.dt.int8,
            kind="ExternalInput"))
    out_pe = nc.dram_tensor("out_pe", [128, PE_F], mybir.dt.float16,
                            kind="ExternalOutput")
    out_dve = nc.dram_tensor("out_dve", [128, DVE_F], mybir.dt.float16,
                             kind="ExternalOutput")

    with (
        TileContext(nc) as tc,
        tc.tile_pool(name="inp", bufs=1) as inp,
        tc.tile_pool(name="stgp", bufs=2) as stgp,
        tc.tile_pool(name="psp", bufs=1, space="PSUM") as psp,
    ):
        lhst = inp.tile([128, 640], mybir.dt.float16)
        pe8 = {g: inp.tile([_pe_rows(*GROUPS[g]), len(_acts_in(g)) * PE_SLAB_F],
                           mybir.dt.int8, name=f"pe8_{g}") for g in range(4)}
        x16 = {t: inp.tile([_pe_rows(*GROUPS[t // 4]), PE_SLAB_F],
                           mybir.dt.float16, name=f"x16_{t}")
               for t in WIDEN}
        x8 = {t: inp.tile([_pe_rows(*GROUPS[t // 4]), PE_SLAB_F],
                          mybir.dt.int8, name=f"x8_{t}")
              for t, route in WIDEN.items() if route == "dve"}
        dve8 = [inp.tile([_dve_rows(GROUPS[k // 2][1]),
                          2 * _dve_len(GROUPS[k // 2][0])],
                         mybir.dt.int8, name=f"dve8_{k}") for k in range(8)]
        acc_d = inp.tile([128, DVE_F], mybir.dt.float16)
        evac = inp.tile([128, PE_F], mybir.dt.float16)
        banks = [psp.tile([128, 512], mybir.dt.float32, name=f"bank{i}")
                 for i in range(8)]
        st16 = {g: stgp.tile([128, len(_acts_in(g)) * PE_SLAB_F],
                             mybir.dt.float16, tag="st16", name=f"st16_{g}")
                for g in range(4)}

        # --- DMAs: sync queue = lhst + DVE chunks; pool SWDGE = PE slabs ---
        nc.sync.dma_start(out=lhst[:, :], in_=lhst_d.ap())
        for g in range(4):
            nc.sync.dma_start(out=dve8[2 * g][:, :], in_=dve_d[2 * g].ap())
            nc.gpsimd.dma_start(out=pe8[g][:, :], in_=pe_act_d[g].ap())
            nc.sync.dma_start(out=dve8[2 * g + 1][:, :],
                              in_=dve_d[2 * g + 1].ap())
            for t in range(4 * g, 4 * g + 4):
                if t in WIDEN:
                    dst = x8[t] if WIDEN[t] == "dve" else x16[t]
                    nc.gpsimd.dma_start(out=dst[:, :], in_=pe_x_d[t].ap())

        nc.vector.memset(acc_d[:, :].bitcast(mybir.dt.uint32), 0)

        # Open all 8 psum banks with zero-lhsT matmuls: writes 0 and sets
        # has_written for every cell, so all real matmuls purely
        # accumulate (no reliance on first-piece partial-clear ordering).
        for i in range(8):
            nc.tensor.matmul(out=banks[i][:, :], lhsT=lhst[:, 512:640],
                             rhs=lhst[:, 0:512], start=True, stop=False)
        # HAM pre-warm: dummy accumulate-zero matmuls fill the otherwise
        # idle PE window before the first cast lands, keeping the array
        # busy past the 3.4us activity window so the real stream starts
        # at the warm clock. Numerically +0.0 per cell.
        for i in range(16):
            nc.tensor.matmul(out=banks[i % 8][:, :], lhsT=lhst[:, 512:640],
                             rhs=lhst[:, 0:512], start=False, stop=False)

        # Open all 8 psum banks: zero-matmul sets has_written everywhere.
        for i in range(8):
            nc.tensor.matmul(out=banks[i][:, :], lhsT=lhst[:, 512:640],
                             rhs=lhst[:, 0:512], start=True, stop=False)

        acc4 = acc_d[:, :].rearrange("p (mu mv q) -> p mu mv q",
                                     mu=8, mv=8, q=32)

        def rhs_for(t):
            gR = _pe_rows(*GROUPS[t // 4])
            if t in WIDEN:
                return x16[t][0:gR, :]
            g = t // 4
            k = _acts_in(g).index(t)
            return st16[g][0:gR, k * PE_SLAB_F : (k + 1) * PE_SLAB_F]

        def emit_cast(t):
            g = t // 4
            gR = _pe_rows(*GROUPS[g])
            if WIDEN.get(t) == "dve":
                nc.vector.tensor_copy(out=x16[t][0:gR, :], in_=x8[t][0:gR, :])
            elif t not in WIDEN:
                k = _acts_in(g).index(t)
                sl = slice(k * PE_SLAB_F, (k + 1) * PE_SLAB_F)
                nc.scalar.copy(out=st16[g][0:gR, sl], in_=pe8[g][0:gR, sl])

        def emit_add(t):
            e, f = GROUPS[t // 4]
            a, bb = AB[t % 4]
            dR = _dve_rows(f)
            dvv = dve8[t // 2][0:dR,
                               (t % 2) * _dve_len(e) : (t % 2 + 1) * _dve_len(e)]
            dvv = dvv.rearrange("p (iu iv q) -> p iu iv q",
                                iu=7, iv=7, q=(8 - e) * 4)
            ov = acc4[0:dR, a : a + 7, bb : bb + 7, 4 * e : 32]
            nc.vector.tensor_add(out=ov, in0=ov, in1=dvv)

        rvs = {}

        def emit_mm(pi):
            t, iu, m, bb = _PIECES[pi]
            if t not in rvs:
                rvs[t] = rhs_for(t).rearrange("p (iu q) -> p iu q", iu=7, q=448)
            g = t // 4
            gR = _pe_rows(*GROUPS[g])
            nc.tensor.matmul(
                out=banks[m][:, bb * 64 : bb * 64 + 448],
                lhsT=lhst[0:gR, 128 * g : 128 * g + 128],
                rhs=rvs[t][:, iu],
                start=False, stop=_BANK_LAST[m] == pi)

        # groups 0-2: term-major
        pi = 0
        for g in range(3):
            for t in range(4 * g, 4 * g + 4):
                emit_cast(t)
                for _ in range(7):
                    emit_mm(pi)
                    pi += 1
                emit_add(t)
        # group 3: casts/copies first, then bank-major matmuls (+ evacs),
        # adds interleaved
        for t in range(12, 16):
            emit_cast(t)
        adds_left = list(range(12, 16))
        while pi < len(_PIECES):
            emit_mm(pi)
            pi += 1
            if adds_left and pi % 4 == 0:
                emit_add(adds_left.pop(0))
        for t in adds_left:
            emit_add(t)

        # --- tail: evacuate banks after the full matmul stream,
        # Act and DVE in parallel (4 banks each) ---
        nc.sync.dma_start(out=out_dve.ap(), in_=acc_d[:, :])
        for m in range(4):
            nc.scalar.copy(out=evac[:, 512 * m : 512 * m + 512],
                           in_=banks[m][:, :])
            nc.vector.tensor_copy(out=evac[:, 2048 + 512 * m : 2560 + 512 * m],
                                  in_=banks[4 + m][:, :])
        nc.sync.dma_start(out=out_pe.ap()[:, 0:2048], in_=evac[:, 0:2048])
        nc.scalar.dma_start(out=out_pe.ap()[:, 2048:4096],
                            in_=evac[:, 2048:4096])
    nc.compile()
    return nc


def kernel(x):
    x = np.ascontiguousarray(np.asarray(x), dtype=np.float32)
    in_maps, scales = _shard(x)
    nc = build_nc()
    from concourse.bass_utils import run_bass_kernel_spmd

    res = run_bass_kernel_spmd(nc, in_maps, core_ids=list(range(8)))
    return _assemble(res.results, scales)
